# revision 13
# baseline (speedup 1.0000x reference)
"""BitLinear forward on 8 Trainium2 NeuronCores (raw Bass implementation).

Math (reference, with EPS-clamped per-token scale xs = clip(mean|x|, EPS)):
    out = ((x / xs) @ sign(w).T + bias) * mean|w| * xs * scale
        = (x @ sign(w).T) * (mean|w| * scale) + bias * (mean|w| * scale * xs)

The xs normalize/denormalize cancels exactly on the matmul term (clamp
included: (x/clip(s))*clip(s) == x), so the heavy path is a sign-binarized
matmul scaled by the scalar c = mean|w| * scale.  The bias term (zero for the
graded input) is also computed on device when bias != 0.

Distribution: pure data-parallel over the 8192 tokens -- each of the 8 cores
computes 1024 rows against the full (replicated) weight.  No collectives;
mean|w| is computed redundantly per core.

Precision: x is split as x = hi + lo with hi = fp16(x), lo = fp16(x - hi);
sign(w) is exact in fp16.  Both halves accumulate into the same fp32 PSUM
group (32 matmuls per output block).  Measured end-to-end error vs the fp32
reference: ~3.5e-7 relative l2 (the fp32 noise floor itself).

This toolchain's walrus rejects >1 sync-wait per engine instruction, which
rules out the Tile scheduler entirely (even its kernel-tail drain violates
that).  So the kernel is raw Bass: five explicit engine programs synced by
explicit semaphores, with every wait emitted as its own instruction.

Engine layout per core (rows=1024, k=2048, o=2048):
  SP    : input DMAs (x slabs, w tiles, c-scalar round trips)
  ACT   : sign(w)->fp16, |w| row-sums (accum), PSUM evictions (*c fold-in)
  DVE   : x hi/lo split, c reduction chain, bias-term ops (bias mode)
  PE    : 32 blocks x 32 matmuls, PSUM bank = row-block, column-major order
  POOL  : output DMAs (separate DMA ring), x row-slab DMAs in bias mode
"""

import sys

sys.path.insert(0, "/opt/trn_rl_repo")

from contextlib import ExitStack

import numpy as np

import concourse.bass as bass
import concourse.mybir as mybir

F32 = mybir.dt.float32
F16 = mybir.dt.float16
AF = mybir.ActivationFunctionType
ALU = mybir.AluOpType
AX = mybir.AxisListType

N_CORES = 8
EPS = 1e-5
P = 128
NT = 512  # output free-dim tile


def build_nc(rows, k, o, with_bias):
    """Per-core kernel: out[rows, o] = (xt.T @ sign(wt)) * c (+ bias term).

    xt:  [k, rows] f32   (x shard, pre-transposed on host)
    wt:  [k, o]    f32   (weight.T, replicated)
    sc:  [1, 1]    f32   (scale)
    bias:[1, o]    f32   (only when with_bias)
    xr:  [rows, k] f32   (row-major x shard; only when with_bias)
    out: [rows, o] f32
    """
    n_m = rows // P          # row blocks (8)
    n_n = o // NT            # output column blocks (4)
    n_ks = k // P            # K subtiles (16)
    n_wt = (k // NT) * n_n   # w tiles of [128, 4, NT] (16)
    n_wkt = k // NT          # w tiles per output column (4)
    NOUT = 4                 # outsb ring slots

    nc = bass.Bass()
    xt = nc.declare_dram_parameter("xt", [k, rows], F32, isOutput=False)
    wt = nc.declare_dram_parameter("wt", [k, o], F32, isOutput=False)
    sc = nc.declare_dram_parameter("sc", [1, 1], F32, isOutput=False)
    if with_bias:
        bias = nc.declare_dram_parameter("bias", [1, o], F32, isOutput=False)
        xr = nc.declare_dram_parameter("xr", [rows, k], F32, isOutput=False)
    out = nc.declare_dram_parameter("out", [rows, o], F32, isOutput=True)
    scr_col = nc.dram_tensor("scr_col", [P], F32)
    scr_c = nc.dram_tensor("scr_c", [1, 1], F32)

    xt_ap = xt[:, :].rearrange("(po pi) f -> pi po f", pi=P)  # [128, n_ks, rows]
    wt_ap = wt[:, :].rearrange("(po pi) f -> pi po f", pi=P)  # [128, n_ks, o]
    out_ap = out[:, :].rearrange("(po pi) f -> pi po f", pi=P)  # [128, n_m, o]
    if with_bias:
        xr_ap = xr[:, :].rearrange("(po pi) f -> pi po f", pi=P)  # [128, n_m, k]

    with ExitStack() as es:
        sem = lambda name: es.enter_context(nc.semaphore(name))
        sb = lambda name, shape, dt=F32: es.enter_context(
            nc.sbuf_tensor(name, shape, dt)
        )
        ps = lambda name: es.enter_context(nc.psum_tensor(name, [P, NT], F32))

        s_wdma = [sem("s_wdma0"), sem("s_wdma1")]  # per wst slot (16/dma)
        s_xdma = [sem("s_xdma0"), sem("s_xdma1")]  # per xst slot (16/dma)
        s_wproc = sem("s_wproc")  # ACT finished sign+abs of tile t (1/tile)
        s_xproc = sem("s_xproc")  # DVE finished hi/lo of slab m (1/slab)
        s_hi = sem("s_hi")        # DVE hi cast done (1/slab)
        s_dvec = sem("s_dvec")    # DVE c-chain step counter
        s_mm = sem("s_mm")        # PE finished block (1/block)
        s_evict = sem("s_evict")  # ACT finished evict (1/block)
        s_odma = [sem(f"s_odma{i}") for i in range(4)]  # per outsb slot
        s_scs = sem("s_scs")      # scale scalar DMA
        s_col = sem("s_col")      # DVE col reduce done
        s_c0 = sem("s_c0")        # col->dram dma
        s_c1 = sem("s_c1")        # dram->rowt dma
        s_cts = sem("s_cts")      # DVE c scalar ready
        s_c2 = sem("s_c2")        # cts->dram dma
        s_cdma = sem("s_cdma")    # cb broadcast dma
        if with_bias:
            s_xrdma = [sem("s_xrdma0"), sem("s_xrdma1")]  # per xrst slot
            s_bb = sem("s_bb")        # bias broadcast DMA
            s_xsr = sem("s_xsr")      # DVE xs reduce done (1/slab)
            s_xs = sem("s_xs")        # DVE xs[m] clipped (1/slab)
            s_bt1 = sem("s_bt1")      # DVE btmp written (1/block)
            s_bterm = sem("s_bterm")  # DVE bias-term added (1/block)

        w16 = sb("w16", [P, n_ks, o], F16)
        xhi = sb("xhi", [P, n_ks, rows], F16)
        xlo = sb("xlo", [P, n_ks, rows], F16)
        wst = sb("wst", [P, 2, NT // P, NT], F32)
        xst = sb("xst", [P, 2, n_ks, P], F32)
        absw = sb("absw", [P, NT // P, NT], F32)
        acc = sb("acc", [P, n_wt], F32)
        outsb = sb("outsb", [P, NOUT, NT], F32)
        scs = sb("scs", [1, 1], F32)
        col = sb("col", [P, 1], F32)
        rowt = sb("rowt", [1, P], F32)
        tot = sb("tot", [1, 1], F32)
        cts = sb("cts", [1, 1], F32)
        cb = sb("cb", [P, 1], F32)
        if with_bias:
            xrst = sb("xrst", [P, 2, k], F32)
            biasb = sb("biasb", [P, o], F32)
            xs = sb("xs", [P, n_m], F32)
            btmp = sb("btmp", [P, 2, NT], F32)
        psum = [ps(f"psum{m}") for m in range(n_m)]

        # w DMA order: n-major (all k-tiles of column 0 first, ...).  The SP
        # issue order interleaves x slabs so PE's column-0 sweep stays fed.
        w_order = [(kt, nt) for nt in range(n_n) for kt in range(n_wkt)]

        with nc.Block() as block:

            @block.sync
            def _(sp):
                # scale scalar first (tiny)
                sp.dma_start(out=scs[:], in_=sc[:, :]).then_inc(s_scs, 16)
                # interleave: x0, x1, wcol0, x2, x3, wcol1, x4, x5, wcol2,
                # x6, x7, wcol3  (plus slot-reuse waits)
                xi = iter(range(n_m))
                wi = 0

                def dma_x(m):
                    if m >= 2:
                        sp.wait_ge(s_xproc, m - 1)
                    sp.dma_start(
                        out=xst[:, m % 2], in_=xt_ap[:, :, m * P : (m + 1) * P]
                    ).then_inc(s_xdma[m % 2], 16)

                def dma_w(t):
                    kt, nt = w_order[t]
                    if t >= 2:
                        sp.wait_ge(s_wproc, t - 1)
                    sp.dma_start(
                        out=wst[:, t % 2],
                        in_=wt_ap[:, kt * (NT // P) : (kt + 1) * (NT // P),
                                  nt * NT : (nt + 1) * NT],
                    ).then_inc(s_wdma[t % 2], 16)

                for m in (0, 1):
                    dma_x(m)
                for nt in range(n_n):
                    for _ in range(n_wkt):
                        dma_w(wi)
                        wi += 1
                    for m in (2 + 2 * nt, 3 + 2 * nt):
                        if m < n_m:
                            dma_x(m)
                # c-scalar round trips
                sp.wait_ge(s_col, 1)
                sp.dma_start(out=scr_col[:], in_=col[:, 0]).then_inc(s_c0, 16)
                sp.wait_ge(s_c0, 16)
                sp.dma_start(out=rowt[:], in_=scr_col[None, :]).then_inc(s_c1, 16)
                sp.wait_ge(s_cts, 1)
                sp.dma_start(out=scr_c[:, :], in_=cts[:]).then_inc(s_c2, 16)
                sp.wait_ge(s_c2, 16)
                sp.dma_start(
                    out=cb[:], in_=scr_c[:, :].to_broadcast([P, 1])
                ).then_inc(s_cdma, 16)

            @block.scalar
            def _(act):
                # sign + |w| accumulation, one staged tile at a time
                for t in range(n_wt):
                    kt, nt = w_order[t]
                    act.wait_ge(s_wdma[t % 2], 16 * (t // 2 + 1))
                    act.activation(
                        w16[:, kt * (NT // P) : (kt + 1) * (NT // P),
                            nt * NT : (nt + 1) * NT],
                        wst[:, t % 2],
                        AF.Sign,
                    )
                    if t >= 1:
                        act.wait_ge(s_wproc, t)  # WAW on absw scratch
                    act.activation(
                        absw[:], wst[:, t % 2], AF.Abs,
                        accum_out=acc[:, t : t + 1],
                    ).then_inc(s_wproc, 1)
                # c must be in SBUF before evictions
                act.wait_ge(s_cdma, 16)
                # evictions: outsb = psum * c
                for idx in range(n_n * n_m):
                    nt, m = divmod(idx, n_m)
                    act.wait_ge(s_mm, idx + 1)
                    if idx >= NOUT:
                        act.wait_ge(s_odma[idx % NOUT], 16 * (idx // NOUT))
                    act.activation(
                        outsb[:, idx % NOUT], psum[m][:], AF.Copy,
                        scale=cb[:],
                    ).then_inc(s_evict, 1)

            @block.vector
            def _(dve):
                # x hi/lo split per slab
                for m in range(n_m):
                    dve.wait_ge(s_xdma[m % 2], 16 * (m // 2 + 1))
                    dve.tensor_copy(
                        out=xhi[:, :, m * P : (m + 1) * P], in_=xst[:, m % 2]
                    ).then_inc(s_hi, 1)
                    dve.wait_ge(s_hi, m + 1)  # RAW on xhi
                    dve.tensor_tensor(
                        out=xlo[:, :, m * P : (m + 1) * P],
                        in0=xst[:, m % 2],
                        in1=xhi[:, :, m * P : (m + 1) * P],
                        op=ALU.subtract,
                    ).then_inc(s_xproc, 1)
                # bias mode: per-row |x| means from row-major slabs
                if with_bias:
                    dve.wait_ge(s_bb, 16)
                    for m in range(n_m):
                        dve.wait_ge(s_xrdma[m % 2], 16 * (m // 2 + 1))
                        dve.tensor_reduce(
                            xs[:, m : m + 1], xrst[:, m % 2], axis=AX.X,
                            op=ALU.add, apply_absolute_value=True,
                        ).then_inc(s_xsr, 1)
                        dve.wait_ge(s_xsr, m + 1)
                        dve.tensor_scalar(
                            xs[:, m : m + 1], xs[:, m : m + 1],
                            1.0 / k, EPS, ALU.mult, ALU.max,
                        ).then_inc(s_xs, 1)
                # c chain: sum|w| -> scalar c
                dve.wait_ge(s_scs, 16)
                dve.wait_ge(s_wproc, n_wt)
                dve.tensor_reduce(
                    col[:], acc[:], axis=AX.X, op=ALU.add
                ).then_inc(s_col, 1)
                dve.wait_ge(s_c1, 16)
                dve.tensor_reduce(
                    tot[:], rowt[:], axis=AX.X, op=ALU.add
                ).then_inc(s_dvec, 1)
                dve.wait_ge(s_dvec, 1)
                dve.tensor_tensor(
                    out=cts[:], in0=tot[:], in1=scs[:], op=ALU.mult
                ).then_inc(s_dvec, 1)
                dve.wait_ge(s_dvec, 2)
                dve.tensor_scalar(
                    cts[:], cts[:], 1.0 / (k * o), None, ALU.mult
                ).then_inc(s_cts, 1)
                # bias mode: add (bias * xs_m * c) to evicted tiles
                if with_bias:
                    dve.wait_ge(s_cdma, 16)
                    for idx in range(n_n * n_m):
                        nt, m = divmod(idx, n_m)
                        dve.wait_ge(s_evict, idx + 1)
                        if idx >= 2:
                            dve.wait_ge(s_bterm, idx - 1)  # WAW on btmp slot
                        dve.tensor_scalar(
                            btmp[:, idx % 2],
                            biasb[:, nt * NT : (nt + 1) * NT],
                            xs[:, m : m + 1],
                            cb[:],
                            ALU.mult,
                            ALU.mult,
                        ).then_inc(s_bt1, 1)
                        dve.wait_ge(s_bt1, idx + 1)  # RAW on btmp
                        dve.tensor_tensor(
                            out=outsb[:, idx % NOUT],
                            in0=outsb[:, idx % NOUT],
                            in1=btmp[:, idx % 2],
                            op=ALU.add,
                        ).then_inc(s_bterm, 1)

            @block.tensor
            def _(pe):
                for idx in range(n_n * n_m):
                    nt, m = divmod(idx, n_m)
                    pe.wait_ge(s_xproc, m + 1)
                    pe.wait_ge(s_wproc, n_wkt * (nt + 1))
                    if nt >= 1:
                        pe.wait_ge(s_evict, (nt - 1) * n_m + m + 1)
                    last = None
                    for half, src in ((0, xhi), (1, xlo)):
                        for ks in range(n_ks):
                            last = pe.matmul(
                                psum[m][:],
                                src[:, ks, m * P : (m + 1) * P],
                                w16[:, ks, nt * NT : (nt + 1) * NT],
                                start=(half == 0 and ks == 0),
                                stop=(half == 1 and ks == n_ks - 1),
                            )
                    last.then_inc(s_mm, 1)

            @block.gpsimd
            def _(gp):
                if with_bias:
                    gp.dma_start(
                        out=biasb[:], in_=bias[:, :].to_broadcast([P, o])
                    ).then_inc(s_bb, 16)
                    for m in range(n_m):
                        if m >= 2:
                            gp.wait_ge(s_xs, m - 1)
                        gp.dma_start(
                            out=xrst[:, m % 2], in_=xr_ap[:, m, :]
                        ).then_inc(s_xrdma[m % 2], 16)
                for idx in range(n_n * n_m):
                    nt, m = divmod(idx, n_m)
                    gp.wait_ge(s_bterm if with_bias else s_evict, idx + 1)
                    gp.dma_start(
                        out=out_ap[:, m, nt * NT : (nt + 1) * NT],
                        in_=outsb[:, idx % NOUT],
                    ).then_inc(s_odma[idx % NOUT], 16)

    return nc


_NC_CACHE = {}


def _get_nc(rows, k, o, with_bias):
    key = (rows, k, o, with_bias)
    if key not in _NC_CACHE:
        _NC_CACHE[key] = build_nc(rows, k, o, with_bias)
    return _NC_CACHE[key]


def _run(x, weight, bias, scale, trace=False, tmpdir=None):
    from concourse.bass_utils import run_bass_kernel_spmd

    x = np.asarray(x, dtype=np.float32)
    weight = np.asarray(weight, dtype=np.float32)
    bias_arr = np.asarray(bias, dtype=np.float32).reshape(-1)
    scale_arr = np.asarray(scale, dtype=np.float32).reshape(1, 1)

    b, s, d_in = x.shape
    d_out = weight.shape[0]
    rows_total = b * s
    rows = rows_total // N_CORES
    with_bias = bool(np.any(bias_arr))

    nc = _get_nc(rows, d_in, d_out, with_bias)

    x2 = x.reshape(rows_total, d_in)
    wtr = np.ascontiguousarray(weight.T)
    in_maps = []
    for i in range(N_CORES):
        shard = x2[i * rows : (i + 1) * rows]
        m = {
            "xt": np.ascontiguousarray(shard.T),
            "wt": wtr,
            "sc": scale_arr,
        }
        if with_bias:
            m["bias"] = bias_arr.reshape(1, d_out)
            m["xr"] = np.ascontiguousarray(shard)
        in_maps.append(m)

    res = run_bass_kernel_spmd(
        nc, in_maps, list(range(N_CORES)), trace=trace, tmpdir=tmpdir
    )
    out = np.concatenate([r["out"] for r in res.results], axis=0)
    return out.reshape(b, s, d_out), res


def kernel(x, weight, bias, scale):
    return _run(x, weight, bias, scale)[0]


# revision 16
# speedup vs baseline: 1.0796x; 1.0796x over previous
"""BitLinear forward on 8 Trainium2 NeuronCores (raw Bass implementation).

Math (reference, with EPS-clamped per-token scale xs = clip(mean|x|, EPS)):
    out = ((x / xs) @ sign(w).T + bias) * mean|w| * xs * scale
        = (x @ sign(w).T) * (mean|w| * scale) + bias * (mean|w| * scale * xs)

The xs normalize/denormalize cancels exactly on the matmul term (clamp
included: (x/clip(s))*clip(s) == x), so the heavy path is a sign-binarized
matmul scaled by the scalar c = mean|w| * scale.  The bias term (zero for the
graded input) is also computed on device when bias != 0.

Distribution: pure data-parallel over the 8192 tokens -- each of the 8 cores
computes 1024 rows against the full (replicated) weight.  No collectives;
mean|w| is computed redundantly per core.

Precision: x is split as x = hi + lo with hi = fp16(x), lo = fp16(x - hi);
sign(w) is exact in fp16.  Both halves accumulate into the same fp32 PSUM
group (32 matmuls per output block).  Measured end-to-end error vs the fp32
reference: ~3.5e-7 relative l2 (the fp32 noise floor itself).

This toolchain's walrus rejects >1 sync-wait per engine instruction, which
rules out the Tile scheduler entirely (even its kernel-tail drain violates
that).  So the kernel is raw Bass: five explicit engine programs synced by
explicit semaphores, with every wait emitted as its own instruction.

Engine layout per core (rows=1024, k=2048, o=2048):
  SP    : input DMAs (x slabs, w tiles, c-scalar round trips)
  ACT   : sign(w)->fp16, |w| row-sums (accum), PSUM evictions (*c fold-in)
  DVE   : x hi/lo split, c reduction chain, bias-term ops (bias mode)
  PE    : 32 blocks x 32 matmuls, PSUM bank = row-block, column-major order
  POOL  : output DMAs (separate DMA ring), x row-slab DMAs in bias mode
"""

import sys

sys.path.insert(0, "/opt/trn_rl_repo")

from contextlib import ExitStack

import numpy as np

import concourse.bass as bass
import concourse.mybir as mybir

F32 = mybir.dt.float32
F16 = mybir.dt.float16
AF = mybir.ActivationFunctionType
ALU = mybir.AluOpType
AX = mybir.AxisListType

N_CORES = 8
EPS = 1e-5
P = 128
NT = 512  # output free-dim tile


def build_nc(rows, k, o, with_bias):
    """Per-core kernel: out[rows, o] = (xt.T @ sign(wt)) * c (+ bias term).

    xt:  [k, rows] f32   (x shard, pre-transposed on host)
    wt:  [k, o]    f32   (weight.T, replicated)
    sc:  [1, 1]    f32   (scale)
    bias:[1, o]    f32   (only when with_bias)
    xr:  [rows, k] f32   (row-major x shard; only when with_bias)
    out: [rows, o] f32
    """
    n_m = rows // P          # row blocks (8)
    n_n = o // NT            # output column blocks (4)
    n_ks = k // P            # K subtiles (16)
    n_wt = (k // NT) * n_n   # w tiles of [128, 4, NT] (16)
    n_wkt = k // NT          # w tiles per output column (4)
    NOUT = 4                 # outsb ring slots

    nc = bass.Bass()
    xt = nc.declare_dram_parameter("xt", [k, rows], F32, isOutput=False)
    wt = nc.declare_dram_parameter("wt", [k, o], F32, isOutput=False)
    sc = nc.declare_dram_parameter("sc", [1, 1], F32, isOutput=False)
    if with_bias:
        bias = nc.declare_dram_parameter("bias", [1, o], F32, isOutput=False)
        xr = nc.declare_dram_parameter("xr", [rows, k], F32, isOutput=False)
    out = nc.declare_dram_parameter("out", [rows, o], F32, isOutput=True)
    scr_col = nc.dram_tensor("scr_col", [P], F32)
    scr_c = nc.dram_tensor("scr_c", [1, 1], F32)

    xt_ap = xt[:, :].rearrange("(po pi) f -> pi po f", pi=P)  # [128, n_ks, rows]
    wt_ap = wt[:, :].rearrange("(po pi) f -> pi po f", pi=P)  # [128, n_ks, o]
    out_ap = out[:, :].rearrange("(po pi) f -> pi po f", pi=P)  # [128, n_m, o]
    if with_bias:
        xr_ap = xr[:, :].rearrange("(po pi) f -> pi po f", pi=P)  # [128, n_m, k]

    with ExitStack() as es:
        sem = lambda name: es.enter_context(nc.semaphore(name))
        sb = lambda name, shape, dt=F32: es.enter_context(
            nc.sbuf_tensor(name, shape, dt)
        )
        ps = lambda name: es.enter_context(nc.psum_tensor(name, [P, NT], F32))

        s_wdma = [sem(f"s_wdma{i}") for i in range(4)]  # per wst slot (16/dma)
        s_sign = sem("s_sign")    # ACT sign of tile t done (1/tile)
        s_xdma = [sem(f"s_xdma{i}") for i in range(3)]  # per xst slot
        s_wproc = sem("s_wproc")  # ACT finished sign+abs of tile t (1/tile)
        s_xproc = sem("s_xproc")  # DVE finished hi/lo of slab m (1/slab)
        s_hi = sem("s_hi")        # DVE hi cast done (1/slab)
        s_dvec = sem("s_dvec")    # DVE c-chain step counter
        s_mm = sem("s_mm")        # PE finished block (1/block)
        s_evict = sem("s_evict")  # ACT finished evict (1/block)
        s_odma = [sem(f"s_odma{i}") for i in range(4)]  # per outsb slot
        s_scs = sem("s_scs")      # scale scalar DMA
        s_col = sem("s_col")      # DVE col reduce done
        s_c0 = sem("s_c0")        # col->dram dma
        s_c1 = sem("s_c1")        # dram->rowt dma
        s_cts = sem("s_cts")      # DVE c scalar ready
        s_c2 = sem("s_c2")        # cts->dram dma
        s_cdma = sem("s_cdma")    # cb broadcast dma
        if with_bias:
            s_xrdma = [sem("s_xrdma0"), sem("s_xrdma1")]  # per xrst slot
            s_bb = sem("s_bb")        # bias broadcast DMA
            s_xsr = sem("s_xsr")      # DVE xs reduce done (1/slab)
            s_xs = sem("s_xs")        # DVE xs[m] clipped (1/slab)
            s_bt1 = sem("s_bt1")      # DVE btmp written (1/block)
            s_bterm = sem("s_bterm")  # DVE bias-term added (1/block)

        w16 = sb("w16", [P, n_ks, o], F16)
        xhi = sb("xhi", [P, n_ks, rows], F16)
        xlo = sb("xlo", [P, n_ks, rows], F16)
        wst = sb("wst", [P, 4, NT // P, NT], F32)
        xst = sb("xst", [P, 3, n_ks, P], F32)
        absw = sb("absw", [P, NT // P, NT], F32)
        acc = sb("acc", [P, n_wt], F32)
        outsb = sb("outsb", [P, NOUT, NT], F32)
        scs = sb("scs", [1, 1], F32)
        col = sb("col", [P, 1], F32)
        rowt = sb("rowt", [1, P], F32)
        tot = sb("tot", [1, 1], F32)
        cts = sb("cts", [1, 1], F32)
        cb = sb("cb", [P, 1], F32)
        if with_bias:
            xrst = sb("xrst", [P, 2, k], F32)
            biasb = sb("biasb", [P, o], F32)
            xs = sb("xs", [P, n_m], F32)
            btmp = sb("btmp", [P, 2, NT], F32)
        psum = [ps(f"psum{m}") for m in range(n_m)]

        # w DMA order: n-major (all k-tiles of column 0 first, ...).  The SP
        # issue order interleaves x slabs so PE's column-0 sweep stays fed.
        w_order = [(kt, nt) for nt in range(n_n) for kt in range(n_wkt)]

        with nc.Block() as block:

            @block.sync
            def _(sp):
                # scale scalar first (tiny)
                sp.dma_start(out=scs[:], in_=sc[:, :]).then_inc(s_scs, 16)
                # x slabs on the SP HW ring (3 staging slots)
                for m in range(n_m):
                    if m >= 3:
                        sp.wait_ge(s_xproc, m - 2)
                    sp.dma_start(
                        out=xst[:, m % 3], in_=xt_ap[:, :, m * P : (m + 1) * P]
                    ).then_inc(s_xdma[m % 3], 16)
                # c-scalar round trips
                sp.wait_ge(s_col, 1)
                sp.dma_start(out=scr_col[:], in_=col[:, 0]).then_inc(s_c0, 16)
                sp.wait_ge(s_c0, 16)
                sp.dma_start(out=rowt[:], in_=scr_col[None, :]).then_inc(s_c1, 16)
                sp.wait_ge(s_cts, 1)
                sp.dma_start(out=scr_c[:, :], in_=cts[:]).then_inc(s_c2, 16)
                sp.wait_ge(s_c2, 16)
                sp.dma_start(
                    out=cb[:], in_=scr_c[:, :].to_broadcast([P, 1])
                ).then_inc(s_cdma, 16)

            @block.scalar
            def _(act):
                # w tile DMAs issued from ACT itself: slot reuse is ACT
                # program order (dma for t+4 issued after abs of t), so the
                # ring never stalls on a cross-engine wait.
                def dma_w(t):
                    kt, nt = w_order[t]
                    act.dma_start(
                        out=wst[:, t % 4],
                        in_=wt_ap[:, kt * (NT // P) : (kt + 1) * (NT // P),
                                  nt * NT : (nt + 1) * NT],
                    ).then_inc(s_wdma[t % 4], 16)

                for t in range(min(4, n_wt)):
                    dma_w(t)
                for t in range(n_wt):
                    kt, nt = w_order[t]
                    act.wait_ge(s_wdma[t % 4], 16 * (t // 4 + 1))
                    act.activation(
                        w16[:, kt * (NT // P) : (kt + 1) * (NT // P),
                            nt * NT : (nt + 1) * NT],
                        wst[:, t % 4],
                        AF.Sign,
                    ).then_inc(s_sign, 1)
                    if t >= 1:
                        act.wait_ge(s_wproc, t)  # WAW on absw scratch
                    act.activation(
                        absw[:], wst[:, t % 4], AF.Abs,
                        accum_out=acc[:, t : t + 1],
                    ).then_inc(s_wproc, 1)
                    if t + 4 < n_wt:
                        dma_w(t + 4)
                # c must be in SBUF before evictions
                act.wait_ge(s_cdma, 16)
                # evictions: outsb = psum * c
                for idx in range(n_n * n_m):
                    nt, m = divmod(idx, n_m)
                    act.wait_ge(s_mm, idx + 1)
                    if idx >= NOUT:
                        act.wait_ge(s_odma[idx % NOUT], 16 * (idx // NOUT))
                    act.activation(
                        outsb[:, idx % NOUT], psum[m][:], AF.Copy,
                        scale=cb[:],
                    ).then_inc(s_evict, 1)

            @block.vector
            def _(dve):
                # x hi/lo split per slab
                for m in range(n_m):
                    dve.wait_ge(s_xdma[m % 3], 16 * (m // 3 + 1))
                    dve.tensor_copy(
                        out=xhi[:, :, m * P : (m + 1) * P], in_=xst[:, m % 3]
                    ).then_inc(s_hi, 1)
                    dve.wait_ge(s_hi, m + 1)  # RAW on xhi
                    dve.tensor_tensor(
                        out=xlo[:, :, m * P : (m + 1) * P],
                        in0=xst[:, m % 3],
                        in1=xhi[:, :, m * P : (m + 1) * P],
                        op=ALU.subtract,
                    ).then_inc(s_xproc, 1)
                # bias mode: per-row |x| means from row-major slabs
                if with_bias:
                    dve.wait_ge(s_bb, 16)
                    for m in range(n_m):
                        dve.wait_ge(s_xrdma[m % 2], 16 * (m // 2 + 1))
                        dve.tensor_reduce(
                            xs[:, m : m + 1], xrst[:, m % 2], axis=AX.X,
                            op=ALU.add, apply_absolute_value=True,
                        ).then_inc(s_xsr, 1)
                        dve.wait_ge(s_xsr, m + 1)
                        dve.tensor_scalar(
                            xs[:, m : m + 1], xs[:, m : m + 1],
                            1.0 / k, EPS, ALU.mult, ALU.max,
                        ).then_inc(s_xs, 1)
                # c chain: sum|w| -> scalar c
                dve.wait_ge(s_scs, 16)
                dve.wait_ge(s_wproc, n_wt)
                dve.tensor_reduce(
                    col[:], acc[:], axis=AX.X, op=ALU.add
                ).then_inc(s_col, 1)
                dve.wait_ge(s_c1, 16)
                dve.tensor_reduce(
                    tot[:], rowt[:], axis=AX.X, op=ALU.add
                ).then_inc(s_dvec, 1)
                dve.wait_ge(s_dvec, 1)
                dve.tensor_tensor(
                    out=cts[:], in0=tot[:], in1=scs[:], op=ALU.mult
                ).then_inc(s_dvec, 1)
                dve.wait_ge(s_dvec, 2)
                dve.tensor_scalar(
                    cts[:], cts[:], 1.0 / (k * o), None, ALU.mult
                ).then_inc(s_cts, 1)
                # bias mode: add (bias * xs_m * c) to evicted tiles
                if with_bias:
                    dve.wait_ge(s_cdma, 16)
                    for idx in range(n_n * n_m):
                        nt, m = divmod(idx, n_m)
                        dve.wait_ge(s_evict, idx + 1)
                        if idx >= 2:
                            dve.wait_ge(s_bterm, idx - 1)  # WAW on btmp slot
                        dve.tensor_scalar(
                            btmp[:, idx % 2],
                            biasb[:, nt * NT : (nt + 1) * NT],
                            xs[:, m : m + 1],
                            cb[:],
                            ALU.mult,
                            ALU.mult,
                        ).then_inc(s_bt1, 1)
                        dve.wait_ge(s_bt1, idx + 1)  # RAW on btmp
                        dve.tensor_tensor(
                            out=outsb[:, idx % NOUT],
                            in0=outsb[:, idx % NOUT],
                            in1=btmp[:, idx % 2],
                            op=ALU.add,
                        ).then_inc(s_bterm, 1)

            @block.tensor
            def _(pe):
                for idx in range(n_n * n_m):
                    nt, m = divmod(idx, n_m)
                    pe.wait_ge(s_xproc, m + 1)
                    pe.wait_ge(s_sign, n_wkt * (nt + 1))
                    if nt >= 1:
                        pe.wait_ge(s_evict, (nt - 1) * n_m + m + 1)
                    last = None
                    for half, src in ((0, xhi), (1, xlo)):
                        for ks in range(n_ks):
                            last = pe.matmul(
                                psum[m][:],
                                src[:, ks, m * P : (m + 1) * P],
                                w16[:, ks, nt * NT : (nt + 1) * NT],
                                start=(half == 0 and ks == 0),
                                stop=(half == 1 and ks == n_ks - 1),
                            )
                    last.then_inc(s_mm, 1)

            @block.gpsimd
            def _(gp):
                if with_bias:
                    gp.dma_start(
                        out=biasb[:], in_=bias[:, :].to_broadcast([P, o])
                    ).then_inc(s_bb, 16)
                    for m in range(n_m):
                        if m >= 2:
                            gp.wait_ge(s_xs, m - 1)
                        gp.dma_start(
                            out=xrst[:, m % 2], in_=xr_ap[:, m, :]
                        ).then_inc(s_xrdma[m % 2], 16)
                for idx in range(n_n * n_m):
                    nt, m = divmod(idx, n_m)
                    gp.wait_ge(s_bterm if with_bias else s_evict, idx + 1)
                    gp.dma_start(
                        out=out_ap[:, m, nt * NT : (nt + 1) * NT],
                        in_=outsb[:, idx % NOUT],
                    ).then_inc(s_odma[idx % NOUT], 16)

    return nc


_NC_CACHE = {}


def _get_nc(rows, k, o, with_bias):
    key = (rows, k, o, with_bias)
    if key not in _NC_CACHE:
        _NC_CACHE[key] = build_nc(rows, k, o, with_bias)
    return _NC_CACHE[key]


def _run(x, weight, bias, scale, trace=False, tmpdir=None):
    from concourse.bass_utils import run_bass_kernel_spmd

    x = np.asarray(x, dtype=np.float32)
    weight = np.asarray(weight, dtype=np.float32)
    bias_arr = np.asarray(bias, dtype=np.float32).reshape(-1)
    scale_arr = np.asarray(scale, dtype=np.float32).reshape(1, 1)

    b, s, d_in = x.shape
    d_out = weight.shape[0]
    rows_total = b * s
    rows = rows_total // N_CORES
    with_bias = bool(np.any(bias_arr))

    nc = _get_nc(rows, d_in, d_out, with_bias)

    x2 = x.reshape(rows_total, d_in)
    wtr = np.ascontiguousarray(weight.T)
    in_maps = []
    for i in range(N_CORES):
        shard = x2[i * rows : (i + 1) * rows]
        m = {
            "xt": np.ascontiguousarray(shard.T),
            "wt": wtr,
            "sc": scale_arr,
        }
        if with_bias:
            m["bias"] = bias_arr.reshape(1, d_out)
            m["xr"] = np.ascontiguousarray(shard)
        in_maps.append(m)

    res = run_bass_kernel_spmd(
        nc, in_maps, list(range(N_CORES)), trace=trace, tmpdir=tmpdir
    )
    out = np.concatenate([r["out"] for r in res.results], axis=0)
    return out.reshape(b, s, d_out), res


def kernel(x, weight, bias, scale):
    return _run(x, weight, bias, scale)[0]
